# revision 6
# baseline (speedup 1.0000x reference)
"""Bahdanau additive attention kernel for Trainium2, 8 NeuronCores.

Problem (per full input):
  query    [64, 1, 512]  f32
  proj_key [64, 4096, 512] f32
  value    [64, 4096, 1024] f32
  mask     [64, 4096] int32
  W_q      [512, 512] f32
  v_e      [512] f32
Returns (context [64,1,1024], alphas [64,1,4096]).

Math: q = query @ W_q^T; scores[b,l] = v_e . tanh(q[b] + proj_key[b,l]);
      alphas = softmax(mask(scores)); context = alphas @ value.

Sharding: data parallel over batch, 8 batches per core, no collectives.

Per-core kernel design (memory-bound; DMA of proj_key (64MiB) + value
(128MiB) is the roofline at ~560us):
 - proj_key tiles [128L, 512H] are transposed on the TensorEngine into
   [128H, 512L] PSUM tiles so that:
     * the query add becomes a per-partition ACT bias fused into tanh
     * the v_e-weighted H-reduction becomes a PE matmul (stationary v_e)
 - exp on ACT with accum_out produces the softmax denominator for free
 - exp row is transposed back to partitions via tiny K=1 matmuls and used
   as the stationary vector for the value matvec (PSUM-accumulated over
   all 32 L-tiles of a batch)
 - fp32 data is fed to the big matmuls bitcast as float32r (full-rate on
   the PE at N=512)
"""

import sys
import os

sys.path.insert(0, "/opt/trn_rl_repo")

import numpy as np

B, L, H, D = 64, 4096, 512, 1024
NCORES = 8
BL = B // NCORES     # 8 batches per core
G = 8                # L-groups per batch
GL = L // G          # 512 L positions per group
T = GL // 128        # 4 L-tiles of 128 per group
HC = H // 128        # 4 H-chunks
DC = D // 512        # 2 D-halves

USE_F32R = True
PK_BUFS = 4
VAL_BUFS = 4
MB_BUFS = 1
TH_BUFS = 2

_cached = None


def build_nc():
    import concourse.tile as tile
    from concourse import mybir, bacc
    from concourse.masks import make_identity

    f32 = mybir.dt.float32
    f32r = mybir.dt.float32r
    i32 = mybir.dt.int32
    AF = mybir.ActivationFunctionType
    OP = mybir.AluOpType

    # dtype used for the big matmul operands. float32r streams through the
    # PE at 1 row/cycle (vs 4 for plain fp32) when the moving free dim >=
    # 256; walrus requires the *producing* instruction to emit f32r.
    mmdt = f32r if USE_F32R else f32

    def mm_in(ap):
        # bitcast for operands produced by DMA (bytes pass through).
        return ap.bitcast(f32r) if USE_F32R else ap

    nc = bacc.Bacc("TRN2", debug=False, num_devices=NCORES)
    query = nc.declare_dram_parameter("query", [BL, H], f32, isOutput=False)
    proj_key = nc.declare_dram_parameter("proj_key", [BL, L, H], f32, isOutput=False)
    value = nc.declare_dram_parameter("value", [BL, L, D], f32, isOutput=False)
    mask = nc.declare_dram_parameter("mask", [BL, L], i32, isOutput=False)
    W_q = nc.declare_dram_parameter("W_q", [H, H], f32, isOutput=False)
    v_e = nc.declare_dram_parameter("v_e", [H], f32, isOutput=False)
    out_ctx = nc.declare_dram_parameter("out_ctx", [BL, D], f32, isOutput=True)
    out_alphas = nc.declare_dram_parameter("out_alphas", [BL, L], f32, isOutput=True)

    with tile.TileContext(nc) as tc:
        with (
            tc.tile_pool(name="consts", bufs=1) as consts,
            tc.tile_pool(name="pt", bufs=2, space="PSUM") as ptpool,
            tc.tile_pool(name="sc", bufs=2, space="PSUM") as scpool,
            tc.tile_pool(name="et", bufs=2, space="PSUM") as etpool,
            tc.tile_pool(name="cx", bufs=1, space="PSUM") as cxpool,
            tc.tile_pool(name="pk", bufs=PK_BUFS) as pk_pool,
            tc.tile_pool(name="val", bufs=VAL_BUFS) as val_pool,
            tc.tile_pool(name="th", bufs=TH_BUFS) as th_pool,
            tc.tile_pool(name="expb", bufs=2) as exp_pool,
            tc.tile_pool(name="mb", bufs=MB_BUFS) as mb_pool,
            tc.tile_pool(name="sm", bufs=2) as sm_pool,
            tc.tile_pool(name="ets", bufs=2) as ets_pool,
            tc.tile_pool(name="small", bufs=4) as small_pool,
            tc.tile_pool(name="ctxsb", bufs=2) as ctxsb_pool,
        ):
            # ---- constants / setup ----
            identity = consts.tile([128, 128], f32)
            make_identity(nc, identity)
            one11 = consts.tile([1, 1], f32)
            nc.vector.memset(one11, 1.0)

            v_eT = consts.tile([128, HC], mmdt)  # [h within chunk, hc]
            nc.sync.dma_start(out=v_eT, in_=mm_in(v_e[:].rearrange("(h p) -> p h", p=128)))

            # W_q natural: [o within oc, oc, h]
            wq_nat = consts.tile([128, HC, H], f32)
            nc.sync.dma_start(
                out=wq_nat, in_=W_q[:, :].rearrange("(oc p) h -> p oc h", p=128)
            )
            # W_q transposed: [h within hc, hc, o]
            wqT = consts.tile([128, HC, H], f32)
            for hc in range(HC):
                pt = ptpool.tile([128, GL], f32, tag="pt")
                for oc in range(HC):
                    nc.tensor.transpose(
                        pt[:, oc * 128 : (oc + 1) * 128],
                        wq_nat[:, oc, hc * 128 : (hc + 1) * 128],
                        identity,
                    )
                nc.scalar.copy(out=wqT[:, hc, :], in_=pt)

            q_nat = consts.tile([BL, H], f32)
            nc.sync.dma_start(out=q_nat, in_=query[:, :])
            qinT = consts.tile([128, HC, BL], f32)  # [h within hc, hc, b]
            for hc in range(HC):
                ptq = ptpool.tile([128, BL], f32, tag="pt")
                nc.tensor.transpose(
                    ptq,
                    q_nat[0:BL, hc * 128 : (hc + 1) * 128],
                    identity[0:BL, 0:BL],
                )
                nc.scalar.copy(out=qinT[:, hc, :], in_=ptq)

            # projected query, transposed: [o within oc, oc, b]
            qT = consts.tile([128, HC, BL], f32)
            for oc in range(HC):
                qp = ptpool.tile([128, BL], f32, tag="pt")
                for hc in range(HC):
                    nc.tensor.matmul(
                        qp,
                        wqT[:, hc, oc * 128 : (oc + 1) * 128],
                        qinT[:, hc, :],
                        start=(hc == 0),
                        stop=(hc == HC - 1),
                    )
                nc.scalar.copy(out=qT[:, oc, :], in_=qp)

            # ---- main loop over local batches ----
            for b in range(BL):
                # mask -> additive bias: (m - 1) * 1e30  (0 -> -1e30, 1 -> 0)
                mbi = mb_pool.tile([1, L], i32, tag="mbi")
                nc.sync.dma_start(out=mbi, in_=mask[b : b + 1, :])
                mbf = mb_pool.tile([1, L], f32, tag="mbf")
                nc.vector.tensor_copy(out=mbf, in_=mbi)
                nc.vector.tensor_scalar(
                    out=mbf,
                    in0=mbf,
                    scalar1=1.0,
                    scalar2=1e30,
                    op0=OP.subtract,
                    op1=OP.mult,
                )

                exp_b = exp_pool.tile([1, L], f32)
                dnp = small_pool.tile([1, G], f32, tag="dnp")
                ctx_ps = cxpool.tile([1, D], f32)

                for g in range(G):
                    pk_g = pk_pool.tile([128, T, H], f32)
                    nc.sync.dma_start(
                        out=pk_g,
                        in_=proj_key[b, g * GL : (g + 1) * GL, :].rearrange(
                            "(t p) h -> p t h", p=128
                        ),
                    )
                    val_g = val_pool.tile([128, T, D], mmdt)
                    nc.gpsimd.dma_start(
                        out=val_g,
                        in_=mm_in(value[b, g * GL : (g + 1) * GL, :].rearrange(
                            "(t p) d -> p t d", p=128
                        )),
                    )

                    # transpose pk into [h, l] chunks; tanh(x + q) via ACT bias
                    tanhT = th_pool.tile([128, HC, GL], mmdt)
                    for hc in range(HC):
                        pt = ptpool.tile([128, GL], f32, tag="pt")
                        for t in range(T):
                            nc.tensor.transpose(
                                pt[:, t * 128 : (t + 1) * 128],
                                pk_g[:, t, hc * 128 : (hc + 1) * 128],
                                identity,
                            )
                        nc.scalar.activation(
                            out=tanhT[:, hc, :],
                            in_=pt,
                            func=AF.Tanh,
                            bias=qT[:, hc, b : b + 1],
                        )

                    # scores[l] = sum_h v_e[h] * tanhT[h, l]
                    scores = scpool.tile([1, GL], f32, tag="sc")
                    for hc in range(HC):
                        nc.tensor.matmul(
                            scores,
                            v_eT[:, hc : hc + 1],
                            tanhT[:, hc, :],
                            start=(hc == 0),
                            stop=(hc == HC - 1),
                        )

                    # masked scores, exp, denominator contribution
                    sm = sm_pool.tile([1, GL], f32)
                    nc.vector.tensor_tensor(
                        out=sm,
                        in0=scores,
                        in1=mbf[0:1, g * GL : (g + 1) * GL],
                        op=OP.add,
                    )
                    nc.scalar.activation(
                        out=exp_b[0:1, g * GL : (g + 1) * GL],
                        in_=sm,
                        func=AF.Exp,
                        accum_out=dnp[0:1, g : g + 1],
                    )

                    # exp row -> partitions (stationary for the value matvec)
                    expT_ps = etpool.tile([128, T], f32, tag="et")
                    for t in range(T):
                        nc.tensor.matmul(
                            expT_ps[:, t : t + 1],
                            exp_b[0:1, g * GL + t * 128 : g * GL + (t + 1) * 128],
                            one11,
                            start=True,
                            stop=True,
                            skip_group_check=True,
                        )
                    expT = ets_pool.tile([128, T], mmdt)
                    nc.vector.tensor_copy(out=expT, in_=expT_ps)

                    # context accumulation: ctx += exp_l * value[l, :]
                    for t in range(T):
                        for dc in range(DC):
                            nc.tensor.matmul(
                                ctx_ps[0:1, dc * 512 : (dc + 1) * 512],
                                expT[:, t : t + 1],
                                val_g[:, t, dc * 512 : (dc + 1) * 512],
                                start=(g == 0 and t == 0),
                                stop=(g == G - 1 and t == T - 1),
                                skip_group_check=True,
                            )

                # ---- batch tail: normalize ----
                dsum = small_pool.tile([1, 1], f32, tag="dsum")
                nc.vector.reduce_sum(dsum, dnp, axis=mybir.AxisListType.X)
                rec = small_pool.tile([1, 1], f32, tag="rec")
                nc.vector.reciprocal(out=rec, in_=dsum)

                nc.vector.tensor_scalar_mul(exp_b, exp_b, rec[0:1, 0:1])
                nc.gpsimd.dma_start(out=out_alphas[b : b + 1, :], in_=exp_b)

                ctx_sb = ctxsb_pool.tile([1, D], f32)
                nc.vector.tensor_scalar_mul(ctx_sb, ctx_ps, rec[0:1, 0:1])
                nc.gpsimd.dma_start(out=out_ctx[b : b + 1, :], in_=ctx_sb)

    nc.compile()
    return nc


def _get_nc():
    global _cached
    if _cached is None:
        _cached = build_nc()
    return _cached


def make_in_maps(query, proj_key, value, mask, W_q, v_e):
    query = np.ascontiguousarray(np.asarray(query, dtype=np.float32)).reshape(B, H)
    proj_key = np.ascontiguousarray(np.asarray(proj_key, dtype=np.float32))
    value = np.ascontiguousarray(np.asarray(value, dtype=np.float32))
    mask = np.ascontiguousarray(np.asarray(mask, dtype=np.int32))
    W_q = np.ascontiguousarray(np.asarray(W_q, dtype=np.float32))
    v_e = np.ascontiguousarray(np.asarray(v_e, dtype=np.float32))

    in_maps = []
    for c in range(NCORES):
        s = slice(c * BL, (c + 1) * BL)
        in_maps.append(
            {
                "query": query[s],
                "proj_key": proj_key[s],
                "value": value[s],
                "mask": mask[s],
                "W_q": W_q,
                "v_e": v_e,
            }
        )
    return in_maps


def kernel(query, proj_key, value, mask, W_q, v_e):
    from concourse.bass_utils import run_bass_kernel_spmd

    nc = _get_nc()
    in_maps = make_in_maps(query, proj_key, value, mask, W_q, v_e)
    res = run_bass_kernel_spmd(nc, in_maps, core_ids=list(range(NCORES)))
    context = np.concatenate([res.results[c]["out_ctx"] for c in range(NCORES)], axis=0)
    alphas = np.concatenate(
        [res.results[c]["out_alphas"] for c in range(NCORES)], axis=0
    )
    return context.reshape(B, 1, D), alphas.reshape(B, 1, L)


if __name__ == "__main__":
    q = np.random.randn(B, 1, H).astype(np.float32)
    pk = np.random.randn(B, L, H).astype(np.float32)
    v = np.random.randn(B, L, D).astype(np.float32)
    m = np.ones((B, L), dtype=np.int32)
    wq = (np.random.randn(H, H) / np.sqrt(H)).astype(np.float32)
    ve = (np.random.randn(H) / np.sqrt(H)).astype(np.float32)
    ctxv, al = kernel(query=q, proj_key=pk, value=v, mask=m, W_q=wq, v_e=ve)
    print(ctxv.shape, al.shape)
